# revision 2
# baseline (speedup 1.0000x reference)
"""Multi-head graph-attention layer for Trainium2 (8-core SPMD).

The reference computes per-head projections hp = einsum("bnf,hfd->bhnd", h, W),
dense attention scores e = hp @ hp^T, LeakyReLU, softmax over the last axis,
and then multiplies hp by sum_j(softmax(e))_j. The sum of a softmax over its
own normalization axis is identically 1, so the layer's exact mathematical
output is hp itself (concatenated over heads):

    out[b, n, h*64+d] = sum_f h[b,n,f] * W[h,f,d]  =  (h[b] @ Wc)[n, h*64+d]

with Wc[f, h*64+d] = W[h,f,d]. `adj` is unused by the reference and ignored.

Sharding: data-parallel over the batch dim B=8, one graph per NeuronCore.
Each core computes Y[b]^T = (Wc^T @ h[b]^T) as a [256,256] x [256,2048]
matmul with X in float8_e3m4 (scaled x2; the /2 dequant is folded into the
bf16 weights host-side) and W in bf16 -- mixed-dtype matmul verified on HW,
rel err ~1.35e-2 vs the 2e-2 gate. fp8 X halves input DMA bytes.

HW model distilled from traces (per core):
- All 16 DMA engines round-robin a queue's descriptors; one descriptor row
  <=2048B costs one ~80ns packet slot on one engine, so a full-height
  [128,*] DMA costs >=128 slots (~0.64us aggregate) no matter how thin.
  Minimize DMA count and keep rows at 2048B multiples.
- Only two HWDGE queues exist (sync=SP, scalar=Activation). Issue cost is
  ~0.6us per DMA_DIRECT2D (128 descriptors), first-packet latency ~1.3us
  after issue end (DGE fetch), completion-semaphore observation ~0.85us
  after last packet.
- Measured exec window = (first framework const-MEMSET) .. (last teardown
  instruction). ~1.2us preamble tail + ~7.4us full semaphore-file clear
  teardown are fixed framework costs (a trivial kernel measures ~15.7us);
  only the ~middle work window is ours to compress.
- PE clock ramps from ~0.7-1.2GHz to full (~2.4GHz, 512-col matmul
  pipelines at ~216ns) over ~4.5us of sustained activity -> scratch warmup
  matmuls run from body start until the first input chunk lands.

Pipeline (per core):
- One e3m4 input tensor [128, 5120]: cols [0:1024) hold the bf16 W bytes
  (viewed via AP.bitcast for LDWEIGHTS), then per node-range r its Xk0|Xk1
  fp8 blocks. 3 input DMAs: c0=[W|nodes 0:512) (2048B rows), c1=nodes
  [512:1536) (2048B rows), c2=nodes [1536:2048) (1024B rows) -- 384 slots
  total vs 640 for the bf16 baseline. c0/c1/c2 in need-order on sync.
- 6 matmul ranges 256/256/512/512/256/256; PSUM pairs (r0,r1)->bank0,
  r2->bank1, r3->bank2, (r4,r5)->bank3 per m (8 banks exactly). The last
  range computes m1 first so ACT's evict starts before the PE stream ends.
- Evicts fp32->bf16: DVE does m0, ACT does m1, into y [128,4096] bf16.
- 4 output DMAs, all 2048B rows (512 slots = the floor): oA=r0+r1,
  oB=r2, oC=r3 on sync; oD=r4+r5 on scalar -- scalar's queue is already
  warm (it carried nothing; keep oD there so its issue runs in parallel
  with sync's oC and right after ACT's own final evict).
"""

import numpy as np

import concourse.bass as bass
import concourse.mybir as mybir
import concourse.tile as tile
from concourse import bacc
from concourse.bass_utils import run_bass_kernel_spmd

B = 8          # graphs == cores
N = 2048       # nodes per graph
F_IN = 256     # input features (= contraction dim K)
F_OUT = 256    # num_heads * d_head
P = 128        # SBUF/PSUM partitions
KC = 2         # contraction chunks (256 = 2*128)
MC = 2         # output-feature chunks (256 = 2*128)

XSCALE = 2.0   # X quantization scale; /2 folded into W host-side

WBYTES = 2 * 2 * F_OUT     # 1024 e3m4 cols = [128,512] bf16 (Wc k0|k1)
XIN_COLS = WBYTES + 2 * N  # 5120
YT_COLS = 2 * N            # 4096

# (node_c0, width, psum_bank, psum_lo): compute order. (r0,r1) share bank 0,
# (r4,r5) share bank 3.
MM_RANGES = [
    (0, 256, 0, 0),
    (256, 256, 0, 256),
    (512, 512, 1, 0),
    (1024, 512, 2, 0),
    (1536, 256, 3, 0),
    (1792, 256, 3, 256),
]
# input DMA chunk bounds in e3m4 cols; c0 covers W + ranges 0-1, c1 covers
# ranges 2-3, c2 covers ranges 4-5.
CHUNKS = [(0, 2048), (2048, 4096), (4096, 5120)]
# which chunk index covers each range (for documentation; Tile derives the
# actual waits from the APs)
# output DMA groups: (ranges, yt col bounds, engine)
OUT_GROUPS = [
    ((0, 1), 0, 1024, "sync"),
    ((2,), 1024, 2048, "sync"),
    ((3,), 2048, 3072, "sync"),
    ((4, 5), 3072, 4096, "scalar"),
]

# scratch matmul free-dims covering the first input-DMA wait; PE clock ramps
# only under sustained activity, so start as early as possible and stay busy
# until chunk c0 lands (~2.0us after body start).
WARMUPS = [256, 256, 256, 256, 256, 256]

_module_cache = {}

# test.py reads this after calling kernel() to get profile/exec-time info.
LAST_RESULTS = None


def _xcols(r):
    """(k0_col, k1_col) start columns of range r's X blocks in xin."""
    c0, w, _, _ = MM_RANGES[r]
    s = WBYTES + 2 * c0
    return s, s + w


def _ylo(r):
    """start col of range r's [m0|m1] block in yt."""
    return 2 * MM_RANGES[r][0]


def _build_module() -> bass.Bass:
    f8 = mybir.dt.float8e3
    bf16 = mybir.dt.bfloat16

    nc = bacc.Bacc(None, target_bir_lowering=False, enable_partition_id=False)
    xin = nc.dram_tensor("xin", [P, XIN_COLS], f8, kind="ExternalInput")
    yt = nc.dram_tensor("yt", [P, YT_COLS], bf16, kind="ExternalOutput")

    with tile.TileContext(nc) as tc:
        with (
            tc.tile_pool(name="sbpool", bufs=1) as sbpool,
            tc.tile_pool(name="pspool", bufs=1, space="PSUM") as pspool,
        ):
            # Scratch operands for PE warm-up (values irrelevant, but Tile
            # requires a write; DVE memset is quick and DVE is idle here).
            wu = sbpool.tile([P, 256], bf16, name="wu", tag="wu")
            nc.vector.memset(wu[:], 0.0)
            wu_mm = wu[:]

            x_sb = sbpool.tile([P, XIN_COLS], f8, name="x", tag="x")
            y_sb = sbpool.tile([P, YT_COLS], bf16, name="y", tag="y")

            # Input chunk DMAs in need-order on the sync HWDGE queue.
            for lo, hi in CHUNKS:
                nc.sync.dma_start(x_sb[:, lo:hi], xin[:, lo:hi])

            # bf16 view of the W bytes for LDWEIGHTS: [128, 512] =
            # [k0 m0|m1 (256) | k1 m0|m1 (256)]
            w_view = x_sb[:, 0:WBYTES].bitcast(bf16)

            ps = [
                [
                    pspool.tile([P, 512], mybir.dt.float32, name=f"ps{m}_{j}", tag=f"ps{m}_{j}")
                    for j in range(4)
                ]
                for m in range(MC)
            ]

            def ps_slice(m, r):
                _, w, j, lo = MM_RANGES[r]
                return ps[m][j][:, lo : lo + w]

            # PE clock warm-up on scratch data while chunk c0 is in flight
            # (targets ps[1][2]: real accumulation there starts late; Tile's
            # WAW tracking keeps program order).
            for wfree in WARMUPS:
                nc.tensor.matmul(
                    ps[1][2][:, :wfree], wu_mm[:, :P], wu_mm[:, :wfree],
                    start=True, stop=True,
                )

            for r, (c0, w, _, _) in enumerate(MM_RANGES):
                k0c, k1c = _xcols(r)
                # Last range: m1 half first, so ACT's final evict (which
                # gates oD) starts two matmuls earlier; m0's DVE evict
                # overlaps ACT's issue of oD anyway.
                m_order = (1, 0) if r == len(MM_RANGES) - 1 else (0, 1)
                for k, kc in enumerate((k0c, k1c)):
                    for m in m_order:
                        nc.tensor.matmul(
                            ps_slice(m, r),
                            w_view[:, k * F_OUT + m * P : k * F_OUT + (m + 1) * P],
                            x_sb[:, kc : kc + w],
                            start=(k == 0),
                            stop=(k == KC - 1),
                        )
                # Evict fp32 PSUM -> bf16 SBUF: DVE does m0, ACT does m1 in
                # parallel.
                ylo = _ylo(r)
                nc.vector.tensor_copy(y_sb[:, ylo : ylo + w], ps_slice(0, r))
                nc.scalar.copy(y_sb[:, ylo + w : ylo + 2 * w], ps_slice(1, r))

                # Ship completed groups. All blocks are 2048B rows (128
                # descriptor slots each = the DMA floor).
                for ranges, olo, ohi, eng in OUT_GROUPS:
                    if r == max(ranges):
                        e = nc.sync if eng == "sync" else nc.scalar
                        e.dma_start(yt[:, olo:ohi], y_sb[:, olo:ohi])
    nc.compile()
    return nc


def _get_module() -> bass.Bass:
    if "m" not in _module_cache:
        _module_cache["m"] = _build_module()
    return _module_cache["m"]


def kernel(h: np.ndarray, adj: np.ndarray, W: np.ndarray, **_unused) -> np.ndarray:
    global LAST_RESULTS
    import ml_dtypes

    bf16 = ml_dtypes.bfloat16
    e3m4 = ml_dtypes.float8_e3m4
    h = np.asarray(h, dtype=np.float32)
    W = np.asarray(W, dtype=np.float32)
    # Wc[f, head*64+d] = W[head, f, d]; /XSCALE dequant folded in.
    wc = np.ascontiguousarray(W.transpose(1, 0, 2).reshape(F_IN, F_OUT)) / XSCALE
    wq = np.empty((P, 2 * F_OUT), dtype=bf16)   # [128, 512] = k0|k1
    wq[:, 0:F_OUT] = wc[0:P].astype(bf16)
    wq[:, F_OUT : 2 * F_OUT] = wc[P : 2 * P].astype(bf16)
    wbytes = wq.view(np.uint8)                  # [128, 1024]

    in_maps = []
    for b in range(B):
        xt = (XSCALE * h[b].T).astype(e3m4)     # [256 f, 2048 n]
        xin = np.empty((P, XIN_COLS), dtype=e3m4)
        xin[:, 0:WBYTES] = wbytes.view(e3m4)
        for r, (c0, w, _, _) in enumerate(MM_RANGES):
            s, s1 = _xcols(r)
            xin[:, s : s + w] = xt[0:P, c0 : c0 + w]
            xin[:, s1 : s1 + w] = xt[P : 2 * P, c0 : c0 + w]
        in_maps.append({"xin": xin})

    nc = _get_module()
    res = run_bass_kernel_spmd(nc, in_maps, core_ids=list(range(B)))
    LAST_RESULTS = res

    out = np.empty((B, N, F_OUT), dtype=np.float32)
    yt_full = np.empty((F_OUT, N), dtype=np.float32)
    for b in range(B):
        ytb = res.results[b]["yt"]
        for r, (c0, w, _, _) in enumerate(MM_RANGES):
            ylo = _ylo(r)
            blk = ytb[:, ylo : ylo + 2 * w].astype(np.float32)
            yt_full[0:P, c0 : c0 + w] = blk[:, 0:w]
            yt_full[P : 2 * P, c0 : c0 + w] = blk[:, w : 2 * w]
        out[b] = yt_full.T
    return out


# revision 5
# speedup vs baseline: 1.0056x; 1.0056x over previous
"""Multi-head graph-attention layer for Trainium2 (8-core SPMD).

The reference computes per-head projections hp = einsum("bnf,hfd->bhnd", h, W),
dense attention scores e = hp @ hp^T, LeakyReLU, softmax over the last axis,
and then multiplies hp by sum_j(softmax(e))_j. The sum of a softmax over its
own normalization axis is identically 1, so the layer's exact mathematical
output is hp itself (concatenated over heads):

    out[b, n, h*64+d] = sum_f h[b,n,f] * W[h,f,d]  =  (h[b] @ Wc)[n, h*64+d]

with Wc[f, h*64+d] = W[h,f,d]. `adj` is unused by the reference and ignored.

Sharding: data-parallel over the batch dim B=8, one graph per NeuronCore.
Each core computes Y[b]^T = (Wc^T @ h[b]^T) as a [256,256] x [256,2048]
matmul with X in float8_e3m4 (scaled x2; the /2 dequant is folded into the
bf16 weights host-side) and W in bf16 -- mixed-dtype matmul verified on HW,
rel err ~1.35e-2 vs the 2e-2 gate. fp8 X halves input DMA bytes.

HW model distilled from traces (per core):
- All 16 DMA engines round-robin a queue's descriptors; one descriptor row
  <=2048B costs one ~80ns packet slot on one engine, so a full-height
  [128,*] DMA costs >=128 slots (~0.64us aggregate) no matter how thin.
  Minimize DMA count and keep rows at 2048B multiples.
- Only two HWDGE queues exist (sync=SP, scalar=Activation). Issue cost is
  ~0.6us per DMA_DIRECT2D (128 descriptors), first-packet latency ~1.3us
  after issue end (DGE fetch), completion-semaphore observation ~0.85us
  after last packet.
- Measured exec window = (first framework const-MEMSET) .. (last teardown
  instruction). ~1.2us preamble tail + ~7.4us full semaphore-file clear
  teardown are fixed framework costs (a trivial kernel measures ~15.7us);
  only the ~middle work window is ours to compress.
- PE clock ramps from ~0.7-1.2GHz to full (~2.4GHz, 512-col matmul
  pipelines at ~216ns) over ~4.5us of sustained activity -> scratch warmup
  matmuls run from body start until the first input chunk lands.

Pipeline (per core):
- One e3m4 input tensor [128, 5120]: cols [0:1024) hold the bf16 W bytes
  (viewed via AP.bitcast for LDWEIGHTS), then per node-range r its Xk0|Xk1
  fp8 blocks. 3 input DMAs: c0=[W|nodes 0:512) (2048B rows), c1=nodes
  [512:1536) (2048B rows), c2=nodes [1536:2048) (1024B rows) -- 384 slots
  total vs 640 for the bf16 baseline. c0/c1/c2 in need-order on sync.
- 6 matmul ranges 256/256/512/512/256/256; PSUM pairs (r0,r1)->bank0,
  r2->bank1, r3->bank2, (r4,r5)->bank3 per m (8 banks exactly). The last
  range computes m1 first so ACT's evict starts before the PE stream ends.
- Evicts fp32->bf16: DVE does m0, ACT does m1, into y [128,4096] bf16.
- 4 output DMAs, all 2048B rows (512 slots = the floor): oA=r0+r1,
  oB=r2, oC=r3 on sync; oD=r4+r5 on scalar -- scalar's queue is already
  warm (it carried nothing; keep oD there so its issue runs in parallel
  with sync's oC and right after ACT's own final evict).
"""

import numpy as np

import concourse.bass as bass
import concourse.mybir as mybir
import concourse.tile as tile
from concourse import bacc
from concourse.bass_utils import run_bass_kernel_spmd

B = 8          # graphs == cores
N = 2048       # nodes per graph
F_IN = 256     # input features (= contraction dim K)
F_OUT = 256    # num_heads * d_head
P = 128        # SBUF/PSUM partitions
KC = 2         # contraction chunks (256 = 2*128)
MC = 2         # output-feature chunks (256 = 2*128)

XSCALE = 2.0   # X quantization scale; /2 folded into W host-side

WBYTES = 2 * 2 * F_OUT     # 1024 e3m4 cols = [128,512] bf16 (Wc k0|k1)
XIN_COLS = WBYTES + 2 * N  # 5120
YT_COLS = 2 * N            # 4096

# (node_c0, width, psum_bank, psum_lo): compute order. Bank sharing pairs
# DISTANT ranges (r0<->r4, r1<->r5): Tile tracks PSUM WAR at bank
# granularity, so a range sharing a bank with its neighbor would stall its
# matmuls on the neighbor's evicts (measured: the scheduler scrambled the
# whole range order and slid the tail ~2.5us).
MM_RANGES = [
    (0, 256, 0, 0),
    (256, 256, 1, 0),
    (512, 512, 2, 0),
    (1024, 512, 3, 0),
    (1536, 256, 0, 256),
    (1792, 256, 1, 256),
]
# input DMA chunk bounds in e3m4 cols; c0 covers W + ranges 0-1, c1 covers
# ranges 2-3, c2 covers ranges 4-5.
CHUNKS = [(0, 2048), (2048, 4096), (4096, 5120)]
# which chunk index covers each range (for documentation; Tile derives the
# actual waits from the APs)
# output DMA groups: (ranges, yt col bounds, engine)
OUT_GROUPS = [
    ((0, 1), 0, 1024, "sync"),
    ((2,), 1024, 2048, "sync"),
    ((3,), 2048, 3072, "sync"),
    ((4, 5), 3072, 4096, "scalar"),
]

# scratch matmul free-dims covering the first input-DMA wait; PE clock ramps
# only under sustained activity, so start as early as possible and stay busy
# until chunk c0's completion semaphore is observable (~2.9us after body
# start: 0.6 issue + 0.75 DGE + ~1.1 ramped transfer + 0.85 sem latency).
WARMUPS = [256, 256, 256, 256, 256, 256, 256]

_module_cache = {}

# test.py reads this after calling kernel() to get profile/exec-time info.
LAST_RESULTS = None


def _xcols(r):
    """(k0_col, k1_col) start columns of range r's X blocks in xin."""
    c0, w, _, _ = MM_RANGES[r]
    s = WBYTES + 2 * c0
    return s, s + w


def _ylo(r):
    """start col of range r's [m0|m1] block in yt."""
    return 2 * MM_RANGES[r][0]


def _build_module() -> bass.Bass:
    f8 = mybir.dt.float8e3
    bf16 = mybir.dt.bfloat16

    nc = bacc.Bacc(None, target_bir_lowering=False, enable_partition_id=False)
    xin = nc.dram_tensor("xin", [P, XIN_COLS], f8, kind="ExternalInput")
    yt = nc.dram_tensor("yt", [P, YT_COLS], bf16, kind="ExternalOutput")

    with tile.TileContext(nc) as tc:
        with (
            tc.tile_pool(name="sbpool", bufs=1) as sbpool,
            tc.tile_pool(name="pspool", bufs=1, space="PSUM") as pspool,
        ):
            # Scratch operands for PE warm-up (values irrelevant, but Tile
            # requires a write; DVE memset is quick and DVE is idle here).
            wu = sbpool.tile([P, 256], bf16, name="wu", tag="wu")
            nc.vector.memset(wu[:], 0.0)
            wu_mm = wu[:]

            x_sb = sbpool.tile([P, XIN_COLS], f8, name="x", tag="x")
            y_sb = sbpool.tile([P, YT_COLS], bf16, name="y", tag="y")

            # Input chunk DMAs in need-order on the sync HWDGE queue.
            for lo, hi in CHUNKS:
                nc.sync.dma_start(x_sb[:, lo:hi], xin[:, lo:hi])

            # bf16 view of the W bytes for LDWEIGHTS: [128, 512] =
            # [k0 m0|m1 (256) | k1 m0|m1 (256)]
            w_view = x_sb[:, 0:WBYTES].bitcast(bf16)

            ps = [
                [
                    pspool.tile([P, 512], mybir.dt.float32, name=f"ps{m}_{j}", tag=f"ps{m}_{j}")
                    for j in range(4)
                ]
                for m in range(MC)
            ]

            def ps_slice(m, r):
                _, w, j, lo = MM_RANGES[r]
                return ps[m][j][:, lo : lo + w]

            # PE clock warm-up on scratch data while chunk c0 is in flight
            # (targets ps[1][3]: real accumulation there starts last; Tile's
            # WAW tracking keeps program order).
            for wfree in WARMUPS:
                nc.tensor.matmul(
                    ps[1][3][:, :wfree], wu_mm[:, :P], wu_mm[:, :wfree],
                    start=True, stop=True,
                )

            for r, (c0, w, _, _) in enumerate(MM_RANGES):
                k0c, k1c = _xcols(r)
                # Last range: m1 half first, so ACT's final evict (which
                # gates oD) starts two matmuls earlier; m0's DVE evict
                # overlaps ACT's issue of oD anyway.
                m_order = (1, 0) if r == len(MM_RANGES) - 1 else (0, 1)
                for k, kc in enumerate((k0c, k1c)):
                    for m in m_order:
                        nc.tensor.matmul(
                            ps_slice(m, r),
                            w_view[:, k * F_OUT + m * P : k * F_OUT + (m + 1) * P],
                            x_sb[:, kc : kc + w],
                            start=(k == 0),
                            stop=(k == KC - 1),
                        )
                # Evict fp32 PSUM -> bf16 SBUF: DVE does m0, ACT does m1 in
                # parallel.
                ylo = _ylo(r)
                nc.vector.tensor_copy(y_sb[:, ylo : ylo + w], ps_slice(0, r))
                nc.scalar.copy(y_sb[:, ylo + w : ylo + 2 * w], ps_slice(1, r))

                # Ship completed groups. All blocks are 2048B rows (128
                # descriptor slots each = the DMA floor).
                for ranges, olo, ohi, eng in OUT_GROUPS:
                    if r == max(ranges):
                        e = nc.sync if eng == "sync" else nc.scalar
                        e.dma_start(yt[:, olo:ohi], y_sb[:, olo:ohi])
    nc.compile()
    return nc


def _get_module() -> bass.Bass:
    if "m" not in _module_cache:
        _module_cache["m"] = _build_module()
    return _module_cache["m"]


def kernel(h: np.ndarray, adj: np.ndarray, W: np.ndarray, **_unused) -> np.ndarray:
    global LAST_RESULTS
    import ml_dtypes

    bf16 = ml_dtypes.bfloat16
    e3m4 = ml_dtypes.float8_e3m4
    h = np.asarray(h, dtype=np.float32)
    W = np.asarray(W, dtype=np.float32)
    # Wc[f, head*64+d] = W[head, f, d]; /XSCALE dequant folded in.
    wc = np.ascontiguousarray(W.transpose(1, 0, 2).reshape(F_IN, F_OUT)) / XSCALE
    wq = np.empty((P, 2 * F_OUT), dtype=bf16)   # [128, 512] = k0|k1
    wq[:, 0:F_OUT] = wc[0:P].astype(bf16)
    wq[:, F_OUT : 2 * F_OUT] = wc[P : 2 * P].astype(bf16)
    wbytes = wq.view(np.uint8)                  # [128, 1024]

    in_maps = []
    for b in range(B):
        xt = (XSCALE * h[b].T).astype(e3m4)     # [256 f, 2048 n]
        xin = np.empty((P, XIN_COLS), dtype=e3m4)
        xin[:, 0:WBYTES] = wbytes.view(e3m4)
        for r, (c0, w, _, _) in enumerate(MM_RANGES):
            s, s1 = _xcols(r)
            xin[:, s : s + w] = xt[0:P, c0 : c0 + w]
            xin[:, s1 : s1 + w] = xt[P : 2 * P, c0 : c0 + w]
        in_maps.append({"xin": xin})

    nc = _get_module()
    res = run_bass_kernel_spmd(nc, in_maps, core_ids=list(range(B)))
    LAST_RESULTS = res

    out = np.empty((B, N, F_OUT), dtype=np.float32)
    yt_full = np.empty((F_OUT, N), dtype=np.float32)
    for b in range(B):
        ytb = res.results[b]["yt"]
        for r, (c0, w, _, _) in enumerate(MM_RANGES):
            ylo = _ylo(r)
            blk = ytb[:, ylo : ylo + 2 * w].astype(np.float32)
            yt_full[0:P, c0 : c0 + w] = blk[:, 0:w]
            yt_full[P : 2 * P, c0 : c0 + w] = blk[:, w : 2 * w]
        out[b] = yt_full.T
    return out


# revision 9
# speedup vs baseline: 1.0078x; 1.0022x over previous
"""Multi-head graph-attention layer for Trainium2 (8-core SPMD).

The reference computes per-head projections hp = einsum("bnf,hfd->bhnd", h, W),
dense attention scores e = hp @ hp^T, LeakyReLU, softmax over the last axis,
and then multiplies hp by sum_j(softmax(e))_j. The sum of a softmax over its
own normalization axis is identically 1, so the layer's exact mathematical
output is hp itself (concatenated over heads):

    out[b, n, h*64+d] = sum_f h[b,n,f] * W[h,f,d]  =  (h[b] @ Wc)[n, h*64+d]

with Wc[f, h*64+d] = W[h,f,d]. `adj` is unused by the reference and ignored.

Sharding: data-parallel over the batch dim B=8, one graph per NeuronCore.
Each core computes Y[b]^T = (Wc^T @ h[b]^T) as a [256,256] x [256,2048]
matmul with X in float8_e3m4 (scaled x2; the /2 dequant is folded into the
bf16 weights host-side) and W in bf16 -- mixed-dtype matmul verified on HW,
rel err ~1.35e-2 vs the 2e-2 gate. fp8 X halves input DMA bytes.

HW model distilled from traces (per core):
- All 16 DMA engines round-robin a queue's descriptors; one descriptor row
  <=2048B costs one ~80ns packet slot on one engine, so a full-height
  [128,*] DMA costs >=128 slots (~0.64us aggregate) no matter how thin.
  Minimize DMA count and keep rows at 2048B multiples.
- Only two HWDGE queues exist (sync=SP, scalar=Activation). Issue cost is
  ~0.6us per DMA_DIRECT2D (128 descriptors), first-packet latency ~1.3us
  after issue end (DGE fetch), completion-semaphore observation ~0.85us
  after last packet.
- Measured exec window = (first framework const-MEMSET) .. (last teardown
  instruction). ~1.2us preamble tail + ~7.4us full semaphore-file clear
  teardown are fixed framework costs (a trivial kernel measures ~15.7us);
  only the ~middle work window is ours to compress.
- PE clock ramps from ~0.7-1.2GHz to full (~2.4GHz, 512-col matmul
  pipelines at ~216ns) over ~4.5us of sustained activity -> scratch warmup
  matmuls run from body start until the first input chunk lands.

Pipeline (per core):
- One e3m4 input tensor [128, 5120]: cols [0:1024) hold the bf16 W bytes
  (viewed via AP.bitcast for LDWEIGHTS), then per node-range r its Xk0|Xk1
  fp8 blocks. 3 input DMAs: c0=[W|nodes 0:512) (2048B rows), c1=nodes
  [512:1536) (2048B rows), c2=nodes [1536:2048) (1024B rows) -- 384 slots
  total vs 640 for the bf16 baseline. c0/c1/c2 in need-order on sync.
- 6 matmul ranges 256/256/512/512/256/256; PSUM pairs (r0,r1)->bank0,
  r2->bank1, r3->bank2, (r4,r5)->bank3 per m (8 banks exactly). The last
  range computes m1 first so ACT's evict starts before the PE stream ends.
- Evicts fp32->bf16: DVE does m0, ACT does m1, into y [128,4096] bf16.
- 4 output DMAs, all 2048B rows (512 slots = the floor): oA=r0+r1,
  oB=r2, oC=r3 on sync; oD=r4+r5 on scalar -- scalar's queue is already
  warm (it carried nothing; keep oD there so its issue runs in parallel
  with sync's oC and right after ACT's own final evict).
"""

import numpy as np

import concourse.bass as bass
import concourse.mybir as mybir
import concourse.tile as tile
from concourse import bacc
from concourse.bass_utils import run_bass_kernel_spmd

B = 8          # graphs == cores
N = 2048       # nodes per graph
F_IN = 256     # input features (= contraction dim K)
F_OUT = 256    # num_heads * d_head
P = 128        # SBUF/PSUM partitions
KC = 2         # contraction chunks (256 = 2*128)
MC = 2         # output-feature chunks (256 = 2*128)

XSCALE = 2.0   # X quantization scale; /2 folded into W host-side

WBYTES = 2 * 2 * F_OUT     # 1024 e3m4 cols = [128,512] bf16 (Wc k0|k1)
XIN_COLS = WBYTES + 2 * N  # 5120
YT_COLS = 2 * N            # 4096

# (node_c0, width, psum_bank, psum_lo): compute order. Bank sharing pairs
# DISTANT ranges (r0<->r4, r1<->r5): Tile tracks PSUM WAR at bank
# granularity, so a range sharing a bank with its neighbor would stall its
# matmuls on the neighbor's evicts (measured: the scheduler scrambled the
# whole range order and slid the tail ~2.5us).
MM_RANGES = [
    (0, 256, 0, 0),
    (256, 256, 1, 0),
    (512, 512, 2, 0),
    (1024, 512, 3, 0),
    (1536, 256, 0, 256),
    (1792, 256, 1, 256),
]
# input DMA chunk bounds in e3m4 cols; c0 covers W + ranges 0-1, c1 covers
# ranges 2-3, c2 covers ranges 4-5.
CHUNKS = [(0, 2048), (2048, 4096), (4096, 5120)]
# which chunk index covers each range (for documentation; Tile derives the
# actual waits from the APs)
# output DMA groups for the leading ranges: (ranges, yt col bounds), all on
# sync, all 2048B rows. The tail ranges (4,5) use an m-split layout instead:
# yt[3072:3584) = [r4m0|r5m0] (shipped by sync after DVE's final evict) and
# yt[3584:4096) = [r4m1|r5m1] (shipped by scalar right after ACT's own final
# evict) -- each half's issue fires as soon as its OWN evict engine is done,
# in parallel on the two HWDGE queues, instead of one DMA waiting for all 4.
OUT_GROUPS = [
    ((0, 1), 0, 1024),
    ((2,), 1024, 2048),
    ((3,), 2048, 3072),
]
TAIL = (4, 5)
TAIL_M0_LO = 3072   # r4m0 at 3072, r5m0 at 3328
TAIL_M1_LO = 3584   # r4m1 at 3584, r5m1 at 3840

# scratch matmul free-dims covering the first input-DMA wait. The PE clock
# ramp advances with COLUMNS PROCESSED (~7-8k columns to full clock), so any
# PE idle before the real stream both delays the stream and wastes ramp
# budget: warm up back-to-back (cadence ~233ns per 256-free when cold) from
# body start until chunk c0's completion semaphore is observable (~2.9us
# after body start: 0.6 issue + 0.75 DGE + ~1.1 ramped transfer + 0.85 sem).
WARMUPS = [256] * 12

_module_cache = {}

# test.py reads this after calling kernel() to get profile/exec-time info.
LAST_RESULTS = None


def _xcols(r):
    """(k0_col, k1_col) start columns of range r's X blocks in xin."""
    c0, w, _, _ = MM_RANGES[r]
    s = WBYTES + 2 * c0
    return s, s + w


def _ylo(r):
    """start col of range r's [m0|m1] block in yt."""
    return 2 * MM_RANGES[r][0]


def _build_module() -> bass.Bass:
    f8 = mybir.dt.float8e3
    bf16 = mybir.dt.bfloat16

    nc = bacc.Bacc(None, target_bir_lowering=False, enable_partition_id=False)
    xin = nc.dram_tensor("xin", [P, XIN_COLS], f8, kind="ExternalInput")
    yt = nc.dram_tensor("yt", [P, YT_COLS], bf16, kind="ExternalOutput")

    with tile.TileContext(nc) as tc:
        with (
            tc.tile_pool(name="sbpool", bufs=1) as sbpool,
            tc.tile_pool(name="pspool", bufs=1, space="PSUM") as pspool,
        ):
            # Scratch operands for PE warm-up (values irrelevant, but Tile
            # requires a write; DVE memset is quick and DVE is idle here).
            wu = sbpool.tile([P, 256], bf16, name="wu", tag="wu")
            nc.vector.memset(wu[:], 0.0)
            wu_mm = wu[:]

            x_sb = sbpool.tile([P, XIN_COLS], f8, name="x", tag="x")
            y_sb = sbpool.tile([P, YT_COLS], bf16, name="y", tag="y")

            # Input chunk DMAs in need-order on the sync HWDGE queue.
            for lo, hi in CHUNKS:
                nc.sync.dma_start(x_sb[:, lo:hi], xin[:, lo:hi])

            # bf16 view of the W bytes for LDWEIGHTS: [128, 512] =
            # [k0 m0|m1 (256) | k1 m0|m1 (256)]
            w_view = x_sb[:, 0:WBYTES].bitcast(bf16)

            ps = [
                [
                    pspool.tile([P, 512], mybir.dt.float32, name=f"ps{m}_{j}", tag=f"ps{m}_{j}")
                    for j in range(4)
                ]
                for m in range(MC)
            ]

            def ps_slice(m, r):
                _, w, j, lo = MM_RANGES[r]
                return ps[m][j][:, lo : lo + w]

            # PE clock warm-up on scratch data while chunk c0 is in flight
            # (targets ps[1][3]: real accumulation there starts last; Tile's
            # WAW tracking keeps program order).
            for wfree in WARMUPS:
                nc.tensor.matmul(
                    ps[1][3][:, :wfree], wu_mm[:, :P], wu_mm[:, :wfree],
                    start=True, stop=True,
                )

            for r, (c0, w, _, _) in enumerate(MM_RANGES):
                k0c, k1c = _xcols(r)
                # Tail ranges: m1 half first, so ACT's evicts (which gate
                # scalar's tail DMA) start two matmuls earlier; m0's DVE
                # evicts overlap the issue anyway.
                m_order = (1, 0) if r in TAIL else (0, 1)
                for k, kc in enumerate((k0c, k1c)):
                    for m in m_order:
                        nc.tensor.matmul(
                            ps_slice(m, r),
                            w_view[:, k * F_OUT + m * P : k * F_OUT + (m + 1) * P],
                            x_sb[:, kc : kc + w],
                            start=(k == 0),
                            stop=(k == KC - 1),
                        )
                # Evict fp32 PSUM -> bf16 SBUF: DVE does m0, ACT does m1 in
                # parallel. Tail ranges land in the m-split layout.
                if r in TAIL:
                    i = r - TAIL[0]
                    nc.vector.tensor_copy(
                        y_sb[:, TAIL_M0_LO + i * w : TAIL_M0_LO + (i + 1) * w],
                        ps_slice(0, r),
                    )
                    nc.scalar.copy(
                        y_sb[:, TAIL_M1_LO + i * w : TAIL_M1_LO + (i + 1) * w],
                        ps_slice(1, r),
                    )
                else:
                    ylo = _ylo(r)
                    nc.vector.tensor_copy(y_sb[:, ylo : ylo + w], ps_slice(0, r))
                    nc.scalar.copy(y_sb[:, ylo + w : ylo + 2 * w], ps_slice(1, r))

                # Ship completed groups (2048B rows = 128 descriptor slots,
                # the full-height DMA floor).
                for ranges, olo, ohi in OUT_GROUPS:
                    if r == max(ranges):
                        nc.sync.dma_start(yt[:, olo:ohi], y_sb[:, olo:ohi])
                if r == TAIL[-1]:
                    nc.scalar.dma_start(
                        yt[:, TAIL_M1_LO:YT_COLS], y_sb[:, TAIL_M1_LO:YT_COLS]
                    )
                    nc.sync.dma_start(
                        yt[:, TAIL_M0_LO:TAIL_M1_LO], y_sb[:, TAIL_M0_LO:TAIL_M1_LO]
                    )
    nc.compile()
    return nc


def _get_module() -> bass.Bass:
    if "m" not in _module_cache:
        _module_cache["m"] = _build_module()
    return _module_cache["m"]


def kernel(h: np.ndarray, adj: np.ndarray, W: np.ndarray, **_unused) -> np.ndarray:
    global LAST_RESULTS
    import ml_dtypes

    bf16 = ml_dtypes.bfloat16
    e3m4 = ml_dtypes.float8_e3m4
    h = np.asarray(h, dtype=np.float32)
    W = np.asarray(W, dtype=np.float32)
    # Wc[f, head*64+d] = W[head, f, d]; /XSCALE dequant folded in.
    wc = np.ascontiguousarray(W.transpose(1, 0, 2).reshape(F_IN, F_OUT)) / XSCALE
    wq = np.empty((P, 2 * F_OUT), dtype=bf16)   # [128, 512] = k0|k1
    wq[:, 0:F_OUT] = wc[0:P].astype(bf16)
    wq[:, F_OUT : 2 * F_OUT] = wc[P : 2 * P].astype(bf16)
    wbytes = wq.view(np.uint8)                  # [128, 1024]

    in_maps = []
    for b in range(B):
        xt = (XSCALE * h[b].T).astype(e3m4)     # [256 f, 2048 n]
        xin = np.empty((P, XIN_COLS), dtype=e3m4)
        xin[:, 0:WBYTES] = wbytes.view(e3m4)
        for r, (c0, w, _, _) in enumerate(MM_RANGES):
            s, s1 = _xcols(r)
            xin[:, s : s + w] = xt[0:P, c0 : c0 + w]
            xin[:, s1 : s1 + w] = xt[P : 2 * P, c0 : c0 + w]
        in_maps.append({"xin": xin})

    nc = _get_module()
    res = run_bass_kernel_spmd(nc, in_maps, core_ids=list(range(B)))
    LAST_RESULTS = res

    out = np.empty((B, N, F_OUT), dtype=np.float32)
    yt_full = np.empty((F_OUT, N), dtype=np.float32)
    for b in range(B):
        ytb = res.results[b]["yt"]
        for r, (c0, w, _, _) in enumerate(MM_RANGES):
            if r in TAIL:
                i = r - TAIL[0]
                m0 = ytb[:, TAIL_M0_LO + i * w : TAIL_M0_LO + (i + 1) * w]
                m1 = ytb[:, TAIL_M1_LO + i * w : TAIL_M1_LO + (i + 1) * w]
            else:
                ylo = _ylo(r)
                m0 = ytb[:, ylo : ylo + w]
                m1 = ytb[:, ylo + w : ylo + 2 * w]
            yt_full[0:P, c0 : c0 + w] = m0.astype(np.float32)
            yt_full[P : 2 * P, c0 : c0 + w] = m1.astype(np.float32)
        out[b] = yt_full.T
    return out


# revision 11
# speedup vs baseline: 1.1261x; 1.1174x over previous
"""Multi-head graph-attention layer for Trainium2 (8-core SPMD).

The reference computes per-head projections hp = einsum("bnf,hfd->bhnd", h, W),
dense attention scores e = hp @ hp^T, LeakyReLU, softmax over the last axis,
and then multiplies hp by sum_j(softmax(e))_j. The sum of a softmax over its
own normalization axis is identically 1, so the layer's exact mathematical
output is hp itself (concatenated over heads):

    out[b, n, h*64+d] = sum_f h[b,n,f] * W[h,f,d]  =  (h[b] @ Wc)[n, h*64+d]

with Wc[f, h*64+d] = W[h,f,d]. `adj` is unused by the reference and ignored.

Sharding: data-parallel over the batch dim B=8, one graph per NeuronCore.
Each core computes Y[b]^T = (Wc^T @ h[b]^T) as a [256,256] x [256,2048]
matmul with X in float8_e3m4 (scaled x2; the /2 dequant is folded into the
bf16 weights host-side) and W in bf16 -- mixed-dtype matmul verified on HW,
rel err ~1.35e-2 vs the 2e-2 gate. fp8 X halves input DMA bytes.

HW model distilled from traces (per core):
- All 16 DMA engines round-robin a queue's descriptors; one descriptor row
  <=2048B costs one ~80ns packet slot on one engine, so a full-height
  [128,*] DMA costs >=128 slots (~0.64us aggregate, ~1.1us while the
  engines are still ramping early in the kernel) no matter how thin.
  Minimize DMA count and keep rows at 2048B multiples.
- Only two HWDGE queues exist (sync=SP, scalar=Activation). Issue cost is
  ~0.6us per DMA_DIRECT2D (128 descriptors), first-packet latency ~1.3us
  after issue end (DGE fetch), and a DMA's 16 per-engine completion
  increments straggle ~0.7us past its last packet, so an input chunk is
  compute-usable only ~1.8us after its transfer starts.
- Measured exec window = (first framework const-MEMSET) .. (last teardown
  instruction). ~1.2us preamble tail + ~8us full semaphore-file clear
  teardown are fixed framework costs (a trivial kernel measures ~15.7us);
  only the ~middle work window is ours to compress.
- The PE runs under a 4/8 power throttle (~1.2GHz effective; warm 512-col
  matmuls pipeline at ~427ns) that lifts to 8/8 (~216ns) after ~6.8us of
  sustained activity from body start (the `ham` profile section records
  the transition; an idle PE gap delays it) -> scratch warmup matmuls run
  back-to-back from body start until chunk c0's semaphore is observable.
  The lift timing varies +-2us with device power state run-to-run.
- Rejected routes, verified on HW: fp8 DoubleRow (2x PE throughput,
  K=256/pass) works for e4m3 but walrus' birverifier rejects float8e3,
  and e4m3's 3 mantissa bits put X-quantization alone at 2.7e-2 > the
  2e-2 gate; DoublePixel mode is silently ignored (identical cadence);
  gpsimd cannot read PSUM (walrus codegen fails), so evicts must stay on
  DVE+ACT; any scalar ACTIVATE hoists a 1.3us ACT_TABLE_LOAD to the head
  of scalar's stream, so scalar cannot carry an early input DMA.

Pipeline (per core):
- One e3m4 input tensor [128, 5120]: cols [0:1024) hold the bf16 W bytes
  (viewed via AP.bitcast for LDWEIGHTS), then per node-range r its Xk0|Xk1
  fp8 blocks. 3 input DMAs: c0=[W|nodes 0:512) (2048B rows), c1=nodes
  [512:1536) (2048B rows), c2=nodes [1536:2048) (1024B rows) -- 384 slots
  total vs 640 for the bf16 baseline. c0/c1/c2 in need-order on sync.
- 12 warmup matmuls bridge body start to c0-usable with zero PE gap
  (cadence ~213ns throttled; a gap both idles the PE and delays the
  throttle lift).
- 6 matmul ranges 256/256/512/512/256/256; PSUM bank sharing pairs
  DISTANT ranges (r0,r4)->bank0, (r1,r5)->bank1, r2->bank2, r3->bank3
  per m (8 banks exactly): Tile tracks PSUM WAR at bank granularity, so
  adjacent-range sharing stalls a range's matmuls on its neighbor's
  evicts and scrambles the whole schedule. Tail ranges compute m1 first
  so ACT's evicts complete before the PE stream ends.
- Evicts fp32->bf16: DVE does m0, ACT does m1, into y [128,4096] bf16.
- 5 output DMAs: oA=r0+r1, oB=r2, oC=r3 on sync (2048B rows, shipped as
  each group's evicts land, overlapping the remaining PE stream); the
  tail (r4,r5) uses an m-split layout -- yt[3072:3584)=[r4m0|r5m0]
  shipped by sync right after DVE's final evict, yt[3584:4096)=
  [r4m1|r5m1] by scalar right after ACT's own final evict -- so the two
  final issues fire in parallel the moment their own evict engine is
  done instead of one DMA waiting on all four evicts.
"""

import numpy as np

import concourse.bass as bass
import concourse.mybir as mybir
import concourse.tile as tile
from concourse import bacc
from concourse.bass_utils import run_bass_kernel_spmd

B = 8          # graphs == cores
N = 2048       # nodes per graph
F_IN = 256     # input features (= contraction dim K)
F_OUT = 256    # num_heads * d_head
P = 128        # SBUF/PSUM partitions
KC = 2         # contraction chunks (256 = 2*128)
MC = 2         # output-feature chunks (256 = 2*128)

XSCALE = 2.0   # X quantization scale; /2 folded into W host-side

WBYTES = 2 * 2 * F_OUT     # 1024 e3m4 cols = [128,512] bf16 (Wc k0|k1)
XIN_COLS = WBYTES + 2 * N  # 5120
YT_COLS = 2 * N            # 4096

# (node_c0, width, psum_bank, psum_lo): compute order. Bank sharing pairs
# DISTANT ranges (r0<->r4, r1<->r5): Tile tracks PSUM WAR at bank
# granularity, so a range sharing a bank with its neighbor would stall its
# matmuls on the neighbor's evicts (measured: the scheduler scrambled the
# whole range order and slid the tail ~2.5us).
MM_RANGES = [
    (0, 256, 0, 0),
    (256, 256, 1, 0),
    (512, 512, 2, 0),
    (1024, 512, 3, 0),
    (1536, 256, 0, 256),
    (1792, 256, 1, 256),
]
# input DMA chunk bounds in e3m4 cols; c0 covers W + ranges 0-1, c1 covers
# ranges 2-3, c2 covers ranges 4-5.
CHUNKS = [(0, 2048), (2048, 4096), (4096, 5120)]
# which chunk index covers each range (for documentation; Tile derives the
# actual waits from the APs)
# output DMA groups for the leading ranges: (ranges, yt col bounds), all on
# sync, all 2048B rows. The tail ranges (4,5) use an m-split layout instead:
# yt[3072:3584) = [r4m0|r5m0] (shipped by sync after DVE's final evict) and
# yt[3584:4096) = [r4m1|r5m1] (shipped by scalar right after ACT's own final
# evict) -- each half's issue fires as soon as its OWN evict engine is done,
# in parallel on the two HWDGE queues, instead of one DMA waiting for all 4.
OUT_GROUPS = [
    ((0, 1), 0, 1024),
    ((2,), 1024, 2048),
    ((3,), 2048, 3072),
]
TAIL = (4, 5)
TAIL_M0_LO = 3072   # r4m0 at 3072, r5m0 at 3328
TAIL_M1_LO = 3584   # r4m1 at 3584, r5m1 at 3840

# scratch matmul free-dims covering the first input-DMA wait. The PE clock
# ramp advances with COLUMNS PROCESSED (~7-8k columns to full clock), so any
# PE idle before the real stream both delays the stream and wastes ramp
# budget: warm up back-to-back (cadence ~233ns per 256-free when cold) from
# body start until chunk c0's completion semaphore is observable (~2.9us
# after body start: 0.6 issue + 0.75 DGE + ~1.1 ramped transfer + 0.85 sem).
WARMUPS = [256] * 12

_module_cache = {}

# test.py reads this after calling kernel() to get profile/exec-time info.
LAST_RESULTS = None


def _xcols(r):
    """(k0_col, k1_col) start columns of range r's X blocks in xin."""
    c0, w, _, _ = MM_RANGES[r]
    s = WBYTES + 2 * c0
    return s, s + w


def _ylo(r):
    """start col of range r's [m0|m1] block in yt."""
    return 2 * MM_RANGES[r][0]


def _build_module() -> bass.Bass:
    f8 = mybir.dt.float8e3
    bf16 = mybir.dt.bfloat16

    nc = bacc.Bacc(None, target_bir_lowering=False, enable_partition_id=False)
    xin = nc.dram_tensor("xin", [P, XIN_COLS], f8, kind="ExternalInput")
    yt = nc.dram_tensor("yt", [P, YT_COLS], bf16, kind="ExternalOutput")

    with tile.TileContext(nc) as tc:
        with (
            tc.tile_pool(name="sbpool", bufs=1) as sbpool,
            tc.tile_pool(name="pspool", bufs=1, space="PSUM") as pspool,
        ):
            # Scratch operands for PE warm-up (values irrelevant, but Tile
            # requires a write; DVE memset is quick and DVE is idle here).
            wu = sbpool.tile([P, 256], bf16, name="wu", tag="wu")
            nc.vector.memset(wu[:], 0.0)
            wu_mm = wu[:]

            x_sb = sbpool.tile([P, XIN_COLS], f8, name="x", tag="x")
            y_sb = sbpool.tile([P, YT_COLS], bf16, name="y", tag="y")

            # Input chunk DMAs in need-order on the sync HWDGE queue.
            for lo, hi in CHUNKS:
                nc.sync.dma_start(x_sb[:, lo:hi], xin[:, lo:hi])

            # bf16 view of the W bytes for LDWEIGHTS: [128, 512] =
            # [k0 m0|m1 (256) | k1 m0|m1 (256)]
            w_view = x_sb[:, 0:WBYTES].bitcast(bf16)

            ps = [
                [
                    pspool.tile([P, 512], mybir.dt.float32, name=f"ps{m}_{j}", tag=f"ps{m}_{j}")
                    for j in range(4)
                ]
                for m in range(MC)
            ]

            def ps_slice(m, r):
                _, w, j, lo = MM_RANGES[r]
                return ps[m][j][:, lo : lo + w]

            # PE clock warm-up on scratch data while chunk c0 is in flight
            # (targets ps[1][3]: real accumulation there starts last; Tile's
            # WAW tracking keeps program order).
            for wfree in WARMUPS:
                nc.tensor.matmul(
                    ps[1][3][:, :wfree], wu_mm[:, :P], wu_mm[:, :wfree],
                    start=True, stop=True,
                )

            for r, (c0, w, _, _) in enumerate(MM_RANGES):
                k0c, k1c = _xcols(r)
                # Tail ranges: m1 half first, so ACT's evicts (which gate
                # scalar's tail DMA) start two matmuls earlier; m0's DVE
                # evicts overlap the issue anyway.
                m_order = (1, 0) if r in TAIL else (0, 1)
                for k, kc in enumerate((k0c, k1c)):
                    for m in m_order:
                        nc.tensor.matmul(
                            ps_slice(m, r),
                            w_view[:, k * F_OUT + m * P : k * F_OUT + (m + 1) * P],
                            x_sb[:, kc : kc + w],
                            start=(k == 0),
                            stop=(k == KC - 1),
                        )
                # Evict fp32 PSUM -> bf16 SBUF: DVE does m0, ACT does m1 in
                # parallel. Tail ranges land in the m-split layout.
                if r in TAIL:
                    i = r - TAIL[0]
                    nc.vector.tensor_copy(
                        y_sb[:, TAIL_M0_LO + i * w : TAIL_M0_LO + (i + 1) * w],
                        ps_slice(0, r),
                    )
                    nc.scalar.copy(
                        y_sb[:, TAIL_M1_LO + i * w : TAIL_M1_LO + (i + 1) * w],
                        ps_slice(1, r),
                    )
                else:
                    ylo = _ylo(r)
                    nc.vector.tensor_copy(y_sb[:, ylo : ylo + w], ps_slice(0, r))
                    nc.scalar.copy(y_sb[:, ylo + w : ylo + 2 * w], ps_slice(1, r))

                # Ship completed groups (2048B rows = 128 descriptor slots,
                # the full-height DMA floor).
                for ranges, olo, ohi in OUT_GROUPS:
                    if r == max(ranges):
                        nc.sync.dma_start(yt[:, olo:ohi], y_sb[:, olo:ohi])
                if r == TAIL[-1]:
                    nc.scalar.dma_start(
                        yt[:, TAIL_M1_LO:YT_COLS], y_sb[:, TAIL_M1_LO:YT_COLS]
                    )
                    nc.sync.dma_start(
                        yt[:, TAIL_M0_LO:TAIL_M1_LO], y_sb[:, TAIL_M0_LO:TAIL_M1_LO]
                    )
    nc.compile()
    return nc


def _get_module() -> bass.Bass:
    if "m" not in _module_cache:
        _module_cache["m"] = _build_module()
    return _module_cache["m"]


def kernel(h: np.ndarray, adj: np.ndarray, W: np.ndarray, **_unused) -> np.ndarray:
    global LAST_RESULTS
    import ml_dtypes

    bf16 = ml_dtypes.bfloat16
    e3m4 = ml_dtypes.float8_e3m4
    h = np.asarray(h, dtype=np.float32)
    W = np.asarray(W, dtype=np.float32)
    # Wc[f, head*64+d] = W[head, f, d]; /XSCALE dequant folded in.
    wc = np.ascontiguousarray(W.transpose(1, 0, 2).reshape(F_IN, F_OUT)) / XSCALE
    wq = np.empty((P, 2 * F_OUT), dtype=bf16)   # [128, 512] = k0|k1
    wq[:, 0:F_OUT] = wc[0:P].astype(bf16)
    wq[:, F_OUT : 2 * F_OUT] = wc[P : 2 * P].astype(bf16)
    wbytes = wq.view(np.uint8)                  # [128, 1024]

    in_maps = []
    for b in range(B):
        xt = (XSCALE * h[b].T).astype(e3m4)     # [256 f, 2048 n]
        xin = np.empty((P, XIN_COLS), dtype=e3m4)
        xin[:, 0:WBYTES] = wbytes.view(e3m4)
        for r, (c0, w, _, _) in enumerate(MM_RANGES):
            s, s1 = _xcols(r)
            xin[:, s : s + w] = xt[0:P, c0 : c0 + w]
            xin[:, s1 : s1 + w] = xt[P : 2 * P, c0 : c0 + w]
        in_maps.append({"xin": xin})

    nc = _get_module()
    res = run_bass_kernel_spmd(nc, in_maps, core_ids=list(range(B)))
    LAST_RESULTS = res

    out = np.empty((B, N, F_OUT), dtype=np.float32)
    yt_full = np.empty((F_OUT, N), dtype=np.float32)
    for b in range(B):
        ytb = res.results[b]["yt"]
        for r, (c0, w, _, _) in enumerate(MM_RANGES):
            if r in TAIL:
                i = r - TAIL[0]
                m0 = ytb[:, TAIL_M0_LO + i * w : TAIL_M0_LO + (i + 1) * w]
                m1 = ytb[:, TAIL_M1_LO + i * w : TAIL_M1_LO + (i + 1) * w]
            else:
                ylo = _ylo(r)
                m0 = ytb[:, ylo : ylo + w]
                m1 = ytb[:, ylo + w : ylo + 2 * w]
            yt_full[0:P, c0 : c0 + w] = m0.astype(np.float32)
            yt_full[P : 2 * P, c0 : c0 + w] = m1.astype(np.float32)
        out[b] = yt_full.T
    return out
